# revision 7
# baseline (speedup 1.0000x reference)
"""ConvLSTM decoder (2 ConvLSTM layers + top conv) on 8 Trainium2 cores.

Sharding: data-parallel over batch — B=8, one batch element per core,
weights replicated. The T=10 recurrence runs fully on-core.

Layout: images are stored in SBUF as a zero-padded flat row-major strip:
each 64-pixel row padded to 66 cols (1 zero col each side), 64 rows
contiguous, plus 68-col zero margins at both ends. A 3x3 'SAME' conv then
becomes 9 shifted matmuls accumulated in PSUM: for tap (dy,dx) the rhs is
the flat strip shifted by dy*66+dx.

Layer-0 i2h has only 64 input channels (half the PE array). Its 9 taps are
packed into 5 matmul groups of K=128 by stacking x with a shifted copy of
x in partitions 64:128 of two buffers (shift +66 pairs taps {-67,-1},
{-66,0},{-65,1}; shift +2 pairs {65,67}; tap 66 stays K=64).
"""

import numpy as np

B, T, C, H, W = 8, 10, 64, 64, 64
CH = 128
NSTEP = T - 1          # 9 recurrent steps
WP = W + 2             # padded row width
FLAT = H * WP          # 4224
MARG = 68              # >= 67 = max |tap offset|
BUFC = MARG + FLAT + MARG
BASE = MARG
HW = H * W             # 4096

# row chunks (r0, r1): 8x7 rows + 2x4 rows; max matmul N = 7*66 = 462 <= 512
CHUNKS = [(i * 7, i * 7 + 7) for i in range(8)] + [(56, 60), (60, 64)]

TAPS = [(dy, dx) for dy in (-1, 0, 1) for dx in (-1, 0, 1)]

# layer-0 i2h tap packing: (offset_of_group0_tap, paired?) per slot;
# slots 0-2 pair (o, o+66) on xb66; slot 3 pairs (65, 67) on xb2;
# slot 4 is the lone K=64 tap at offset 66 read from xb66[0:64].
L0SLOTS = [(-67, "xb66", True), (-66, "xb66", True), (-65, "xb66", True),
           (65, "xb2", True), (66, "xb66", False)]
# (ky,kx) kernel indices per slot: group0 tap, group1 tap
L0SLOT_KK = [((0, 0), (1, 0)), ((0, 1), (1, 1)), ((0, 2), (1, 2)),
             ((2, 0), (2, 2)), ((2, 1), None)]

MM_DT = "bf16"         # "f32" | "f32r" | "bf16"
LOOP_N = 0             # >0: wrap body in a hardware repeat loop (timing only)

_CACHE = {}


def _np_dt(mybir):
    if MM_DT == "bf16":
        return mybir.dt.bfloat16
    if MM_DT == "f32r":
        return mybir.dt.float32r
    return mybir.dt.float32


def _host_cast(a):
    if MM_DT == "bf16":
        import ml_dtypes
        return np.ascontiguousarray(a.astype(ml_dtypes.bfloat16))
    return np.ascontiguousarray(a.astype(np.float32))


def _prep_w(w):
    # [O, I, 3, 3] -> [I, 9*O]; slice for (tap ti, 128-chunk g): ti*O + g*128
    O, I = w.shape[0], w.shape[1]
    return _host_cast(w.transpose(1, 2, 3, 0).reshape(I, 9 * O))


def _prep_w0(w):
    # [512, 64, 3, 3] -> [128, 5*512] slot-stacked for L0 i2h packing
    O, I = w.shape[0], w.shape[1]
    out = np.zeros((2 * I, 5 * O), np.float32)
    for k, (a, b) in enumerate(L0SLOT_KK):
        out[:I, k * O:(k + 1) * O] = w[:, :, a[0], a[1]].T
        if b is not None:
            out[I:2 * I, k * O:(k + 1) * O] = w[:, :, b[0], b[1]].T
    return _host_cast(out)


def _build():
    import concourse.bass as bass
    import concourse.tile as tile
    from concourse import bacc, mybir

    f32 = mybir.dt.float32
    cdt = _np_dt(mybir)          # matmul-input dtype in SBUF (and DRAM)
    AF = mybir.ActivationFunctionType

    nc = bacc.Bacc("TRN2", target_bir_lowering=False, debug=False,
                   num_devices=8)

    ddt = cdt if MM_DT in ("f32r", "bf16") else f32
    xs_d = nc.dram_tensor("xs", [NSTEP, C, HW], ddt, kind="ExternalInput")
    h0_d = nc.dram_tensor("h0i", [CH, HW], ddt, kind="ExternalInput")
    c0_d = nc.dram_tensor("c0i", [CH, HW], f32, kind="ExternalInput")
    h1_d = nc.dram_tensor("h1i", [CH, HW], ddt, kind="ExternalInput")
    c1_d = nc.dram_tensor("c1i", [CH, HW], f32, kind="ExternalInput")
    w0_d = nc.dram_tensor("w0", [2 * C, 5 * 4 * CH], ddt, kind="ExternalInput")
    u0_d = nc.dram_tensor("u0", [CH, 9 * 4 * CH], ddt, kind="ExternalInput")
    w1_d = nc.dram_tensor("w1", [CH, 9 * 4 * CH], ddt, kind="ExternalInput")
    u1_d = nc.dram_tensor("u1", [CH, 9 * 4 * CH], ddt, kind="ExternalInput")
    wt_d = nc.dram_tensor("wt", [CH, 9 * C], ddt, kind="ExternalInput")
    zz_d = nc.dram_tensor("zz", [CH, BUFC], ddt, kind="ExternalInput")
    b0_d = nc.dram_tensor("b0", [CH, 4], f32, kind="ExternalInput")
    b1_d = nc.dram_tensor("b1", [CH, 4], f32, kind="ExternalInput")
    bt_d = nc.dram_tensor("bt", [C, 1], f32, kind="ExternalInput")
    out_d = nc.dram_tensor("out", [T, C, HW], f32, kind="ExternalOutput")

    def interior(ap_2d, s0, nrow):
        # rows of 64 interior cols at stride 66 starting at flat offset s0
        return ap_2d[:, s0:s0 + nrow * WP].rearrange(
            "p (r c) -> p r c", c=WP)[:, :, 1:1 + W]

    with tile.TileContext(nc) as tc:
        with (
            tc.tile_pool(name="pers", bufs=1) as pers,
            tc.tile_pool(name="ps", bufs=8, space="PSUM") as psp,
            tc.tile_pool(name="gt", bufs=6) as gtp,
            tc.tile_pool(name="osb", bufs=6) as osbp,
        ):
            # --- persistent SBUF residents ---
            w0_t = pers.tile([2 * C, 5 * 4 * CH], cdt, tag="w0")
            u0_t = pers.tile([CH, 9 * 4 * CH], cdt, tag="u0")
            w1_t = pers.tile([CH, 9 * 4 * CH], cdt, tag="w1")
            u1_t = pers.tile([CH, 9 * 4 * CH], cdt, tag="u1")
            wt_t = pers.tile([CH, 9 * C], cdt, tag="wt")
            b0_t = pers.tile([CH, 4], f32, tag="b0")
            b1_t = pers.tile([CH, 4], f32, tag="b1")
            bt_t = pers.tile([C, 1], f32, tag="bt")
            xb66 = pers.tile([2 * C, BUFC], cdt, tag="xb66")
            xb2 = pers.tile([2 * C, BUFC], cdt, tag="xb2")
            h0p = [pers.tile([CH, BUFC], cdt, tag=f"h0p{i}", name=f"h0p{i}")
                   for i in range(2)]
            h1p = [pers.tile([CH, BUFC], cdt, tag=f"h1p{i}", name=f"h1p{i}")
                   for i in range(2)]
            c0_t = pers.tile([CH, HW], f32, tag="c0")
            c1_t = pers.tile([CH, HW], f32, tag="c1")

            for t_, d_ in ((w0_t, w0_d), (u0_t, u0_d), (w1_t, w1_d),
                           (u1_t, u1_d), (wt_t, wt_d), (b0_t, b0_d),
                           (b1_t, b1_d), (bt_t, bt_d)):
                nc.sync.dma_start(t_[:], d_.ap())

            # one-time zero fill (margins/padding stay zero forever; the
            # interiors are fully re-written by DMA/compute every iteration)
            for buf in (xb66, xb2, h0p[0], h0p[1], h1p[0], h1p[1]):
                if MM_DT == "f32r":
                    nc.sync.dma_start(buf[:], zz_d.ap()[:buf.shape[0]])
                else:
                    nc.vector.memset(buf[:], 0.0)

            def init_states():
                nc.sync.dma_start(interior(h0p[0], BASE, H), h0_d.ap())
                nc.sync.dma_start(interior(h1p[0], BASE, H), h1_d.ap())
                nc.sync.dma_start(c0_t[:], c0_d.ap())
                nc.sync.dma_start(c1_t[:], c1_d.ap())

            def l0_xtaps(g):
                # x-side matmul slots for layer 0: (lhs, src, np_rhs, off)
                res = []
                for k, (off, srcn, paired) in enumerate(L0SLOTS):
                    src = xb66 if srcn == "xb66" else xb2
                    o = k * 4 * CH + g * CH
                    kk = 2 * C if paired else C
                    res.append((w0_t[:kk, o:o + CH], src, kk, off))
                return res

    # taps for a standard 9-tap conv operand
            def std_taps(w_t, src, kx, g):
                res = []
                for ti in range(9):
                    dy, dx = TAPS[ti]
                    o = ti * 4 * CH + g * CH
                    res.append((w_t[:kx, o:o + CH], src, kx, dy * WP + dx))
                return res

            def conv_gates(xtaps_fn, hin, wh_t, b_t, c_t, hout, h_first):
                """One ConvLSTM cell; 5-chunk groups share stationary
                weights (5 matmuls per ldweights)."""
                for bi in range(0, len(CHUNKS), 5):
                    pair = CHUNKS[bi:bi + 5]
                    gtiles = [[None] * 4 for _ in pair]
                    for g in range(4):
                        pss = [psp.tile([CH, (r1 - r0) * WP], f32, tag="ps",
                                        name="ps") for (r0, r1) in pair]
                        xt = xtaps_fn(g)
                        ht = std_taps(wh_t, hin, CH, g)
                        taps = (ht + xt) if h_first else (xt + ht)
                        nt = len(taps)
                        for k, (lhs, src, kk, off) in enumerate(taps):
                            for j, (r0, r1) in enumerate(pair):
                                s = BASE + r0 * WP + off
                                cw = (r1 - r0) * WP
                                nc.tensor.matmul(pss[j][:], lhs,
                                                 src[:kk, s:s + cw],
                                                 start=(k == 0),
                                                 stop=(k == nt - 1))
                        for j, (r0, r1) in enumerate(pair):
                            nr = r1 - r0
                            gt = gtp.tile([CH, nr * W], f32, tag=f"g{g}",
                                          name=f"g{g}")
                            func = AF.Tanh if g == 2 else AF.Sigmoid
                            nc.scalar.activation(
                                gt[:].rearrange("p (r c) -> p r c", c=W),
                                pss[j][:].rearrange(
                                    "p (r c) -> p r c", c=WP)[:, :, 1:1 + W],
                                func, bias=b_t[:, g:g + 1])
                            gtiles[j][g] = gt
                    for j, (r0, r1) in enumerate(pair):
                        nr = r1 - r0
                        gi, gf, gg, go = gtiles[j]
                        csl = c_t[:, r0 * W:r1 * W]
                        nc.vector.tensor_mul(gg[:], gi[:], gg[:])   # i*g
                        nc.vector.tensor_mul(csl, gf[:], csl)       # f*c
                        nc.vector.tensor_add(csl, csl, gg[:])       # c
                        nc.scalar.activation(gf[:], csl, AF.Tanh)
                        nc.vector.tensor_mul(
                            interior(hout, BASE + r0 * WP, nr),
                            go[:].rearrange("p (r c) -> p r c", c=W),
                            gf[:].rearrange("p (r c) -> p r c", c=W))

            def conv_top(hin, tout):
                for bi in range(0, len(CHUNKS), 5):
                    pair = CHUNKS[bi:bi + 5]
                    pss = [psp.tile([C, (r1 - r0) * WP], f32, tag="ps",
                                    name="ps") for (r0, r1) in pair]
                    for ti in range(9):
                        dy, dx = TAPS[ti]
                        lhs = wt_t[:, ti * C:(ti + 1) * C]
                        for j, (r0, r1) in enumerate(pair):
                            s = BASE + r0 * WP + dy * WP + dx
                            cw = (r1 - r0) * WP
                            nc.tensor.matmul(pss[j][:], lhs, hin[:, s:s + cw],
                                             start=(ti == 0), stop=(ti == 8))
                    for j, (r0, r1) in enumerate(pair):
                        nr = r1 - r0
                        ot = osbp.tile([C, nr * W], f32, tag="ot", name="ot")
                        nc.scalar.activation(
                            ot[:].rearrange("p (r c) -> p r c", c=W),
                            pss[j][:].rearrange(
                                "p (r c) -> p r c", c=WP)[:, :, 1:1 + W],
                            AF.Identity, bias=bt_t[:, 0:1])
                        nc.sync.dma_start(tout[:, r0 * W:r1 * W], ot[:])

            def load_x(t):
                # x strip into: xb66[0:64]@BASE, xb66[64:128]@BASE-66,
                # xb2[0:64]@BASE, xb2[64:128]@BASE-2
                src = xs_d.ap()[t]
                nc.sync.dma_start(
                    interior(xb66[:C, :], BASE, H), src)
                nc.sync.dma_start(
                    interior(xb66[C:2 * C, :], BASE - 66, H), src)
                nc.sync.dma_start(
                    interior(xb2[:C, :], BASE, H), src)
                nc.sync.dma_start(
                    interior(xb2[C:2 * C, :], BASE - 2, H), src)

            def l1_xtaps_for(h0buf):
                return lambda g: std_taps(w1_t, h0buf, CH, g)

            def body():
                init_states()
                conv_top(h1p[0], out_d.ap()[0])
                for t in range(NSTEP):
                    load_x(t)
                    conv_gates(l0_xtaps, h0p[t % 2], u0_t, b0_t, c0_t,
                               h0p[(t + 1) % 2], h_first=False)
                    conv_gates(l1_xtaps_for(h0p[(t + 1) % 2]), h1p[t % 2],
                               u1_t, b1_t, c1_t, h1p[(t + 1) % 2],
                               h_first=True)
                    conv_top(h1p[(t + 1) % 2], out_d.ap()[t + 1])

            if LOOP_N > 0:
                with tc.For_i(0, LOOP_N, 1):
                    body()
            else:
                body()

    nc.compile()
    return nc


def _get_nc():
    if "nc" not in _CACHE:
        _CACHE["nc"] = _build()
    return _CACHE["nc"]


def kernel(target, h0, c0, h1, c1,
           wi0, bi0, wh0, bh0,
           wi1, bi1, wh1, bh1,
           wtop, btop):
    from concourse.bass_utils import run_bass_kernel_spmd

    nc = _get_nc()

    target = np.asarray(target, np.float32)
    shared = {
        "w0": _prep_w0(np.asarray(wi0, np.float32)),
        "u0": _prep_w(np.asarray(wh0, np.float32)),
        "w1": _prep_w(np.asarray(wi1, np.float32)),
        "u1": _prep_w(np.asarray(wh1, np.float32)),
        "wt": _prep_w(np.asarray(wtop, np.float32)),
        "b0": np.ascontiguousarray(
            (np.asarray(bi0) + np.asarray(bh0)).astype(np.float32)
            .reshape(4, CH).T),
        "b1": np.ascontiguousarray(
            (np.asarray(bi1) + np.asarray(bh1)).astype(np.float32)
            .reshape(4, CH).T),
        "bt": np.asarray(btop, np.float32).reshape(C, 1),
        "zz": _host_cast(np.zeros((CH, BUFC), np.float32)),
    }
    in_maps = []
    for b in range(B):
        m = dict(shared)
        m["xs"] = _host_cast(target[b, :NSTEP].reshape(NSTEP, C, HW))
        m["h0i"] = _host_cast(np.asarray(h0, np.float32)[b].reshape(CH, HW))
        m["c0i"] = np.ascontiguousarray(
            np.asarray(c0, np.float32)[b].reshape(CH, HW))
        m["h1i"] = _host_cast(np.asarray(h1, np.float32)[b].reshape(CH, HW))
        m["c1i"] = np.ascontiguousarray(
            np.asarray(c1, np.float32)[b].reshape(CH, HW))
        in_maps.append(m)

    res = run_bass_kernel_spmd(nc, in_maps, core_ids=list(range(B)))
    out = np.stack([res.results[b]["out"].reshape(T, C, H, W)
                    for b in range(B)])
    return out


# revision 8
# speedup vs baseline: 1.0673x; 1.0673x over previous
"""ConvLSTM decoder (2 ConvLSTM layers + top conv) on 8 Trainium2 cores.

Sharding: data-parallel over batch — B=8, one batch element per core,
weights replicated. The T=10 recurrence runs fully on-core.

Layout: images are stored in SBUF as a zero-padded flat row-major strip:
each 64-pixel row padded to 66 cols (1 zero col each side), 64 rows
contiguous, plus 68-col zero margins at both ends. A 3x3 'SAME' conv then
becomes 9 shifted matmuls accumulated in PSUM: for tap (dy,dx) the rhs is
the flat strip shifted by dy*66+dx.

Layer-0 i2h has only 64 input channels (half the PE array). Its 9 taps are
packed into 5 matmul groups of K=128 by stacking x with a shifted copy of
x in partitions 64:128 of two buffers (shift +66 pairs taps {-67,-1},
{-66,0},{-65,1}; shift +2 pairs {65,67}; tap 66 stays K=64).
"""

import numpy as np

B, T, C, H, W = 8, 10, 64, 64, 64
CH = 128
NSTEP = T - 1          # 9 recurrent steps
WP = W + 2             # padded row width
FLAT = H * WP          # 4224
MARG = 68              # >= 67 = max |tap offset|
BUFC = MARG + FLAT + MARG
BASE = MARG
HW = H * W             # 4096

# row chunks (r0, r1): 8x7 rows + 2x4 rows; max matmul N = 7*66 = 462 <= 512
CHUNKS = [(i * 7, i * 7 + 7) for i in range(8)] + [(56, 60), (60, 64)]

TAPS = [(dy, dx) for dy in (-1, 0, 1) for dx in (-1, 0, 1)]

# layer-0 i2h tap packing: (offset_of_group0_tap, paired?) per slot;
# slots 0-2 pair (o, o+66) on xb66; slot 3 pairs (65, 67) on xb2;
# slot 4 is the lone K=64 tap at offset 66 read from xb66[0:64].
L0SLOTS = [(-67, "xb66", True), (-66, "xb66", True), (-65, "xb66", True),
           (65, "xb2", True), (66, "xb66", False)]
# (ky,kx) kernel indices per slot: group0 tap, group1 tap
L0SLOT_KK = [((0, 0), (1, 0)), ((0, 1), (1, 1)), ((0, 2), (1, 2)),
             ((2, 0), (2, 2)), ((2, 1), None)]

MM_DT = "bf16"         # "f32" | "f32r" | "bf16"
LOOP_N = 0             # >0: wrap body in a hardware repeat loop (timing only)

_CACHE = {}


def _np_dt(mybir):
    if MM_DT == "bf16":
        return mybir.dt.bfloat16
    if MM_DT == "f32r":
        return mybir.dt.float32r
    return mybir.dt.float32


def _host_cast(a):
    if MM_DT == "bf16":
        import ml_dtypes
        return np.ascontiguousarray(a.astype(ml_dtypes.bfloat16))
    return np.ascontiguousarray(a.astype(np.float32))


def _prep_w(w):
    # [O, I, 3, 3] -> [I, 9*O]; slice for (tap ti, 128-chunk g): ti*O + g*128
    O, I = w.shape[0], w.shape[1]
    return _host_cast(w.transpose(1, 2, 3, 0).reshape(I, 9 * O))


def _prep_w0(w):
    # [512, 64, 3, 3] -> [128, 5*512] slot-stacked for L0 i2h packing
    O, I = w.shape[0], w.shape[1]
    out = np.zeros((2 * I, 5 * O), np.float32)
    for k, (a, b) in enumerate(L0SLOT_KK):
        out[:I, k * O:(k + 1) * O] = w[:, :, a[0], a[1]].T
        if b is not None:
            out[I:2 * I, k * O:(k + 1) * O] = w[:, :, b[0], b[1]].T
    return _host_cast(out)


def _build():
    import concourse.bass as bass
    import concourse.tile as tile
    from concourse import bacc, mybir

    f32 = mybir.dt.float32
    cdt = _np_dt(mybir)          # matmul-input dtype in SBUF (and DRAM)
    AF = mybir.ActivationFunctionType

    nc = bacc.Bacc("TRN2", target_bir_lowering=False, debug=False,
                   num_devices=8)

    ddt = cdt if MM_DT in ("f32r", "bf16") else f32
    xs_d = nc.dram_tensor("xs", [NSTEP, C, HW], ddt, kind="ExternalInput")
    h0_d = nc.dram_tensor("h0i", [CH, HW], ddt, kind="ExternalInput")
    c0_d = nc.dram_tensor("c0i", [CH, HW], f32, kind="ExternalInput")
    h1_d = nc.dram_tensor("h1i", [CH, HW], ddt, kind="ExternalInput")
    c1_d = nc.dram_tensor("c1i", [CH, HW], f32, kind="ExternalInput")
    w0_d = nc.dram_tensor("w0", [2 * C, 5 * 4 * CH], ddt, kind="ExternalInput")
    u0_d = nc.dram_tensor("u0", [CH, 9 * 4 * CH], ddt, kind="ExternalInput")
    w1_d = nc.dram_tensor("w1", [CH, 9 * 4 * CH], ddt, kind="ExternalInput")
    u1_d = nc.dram_tensor("u1", [CH, 9 * 4 * CH], ddt, kind="ExternalInput")
    wt_d = nc.dram_tensor("wt", [CH, 9 * C], ddt, kind="ExternalInput")
    zz_d = nc.dram_tensor("zz", [CH, BUFC], ddt, kind="ExternalInput")
    b0_d = nc.dram_tensor("b0", [CH, 4], f32, kind="ExternalInput")
    b1_d = nc.dram_tensor("b1", [CH, 4], f32, kind="ExternalInput")
    bt_d = nc.dram_tensor("bt", [C, 1], f32, kind="ExternalInput")
    out_d = nc.dram_tensor("out", [T, C, HW], f32, kind="ExternalOutput")

    def interior(ap_2d, s0, nrow):
        # rows of 64 interior cols at stride 66 starting at flat offset s0
        return ap_2d[:, s0:s0 + nrow * WP].rearrange(
            "p (r c) -> p r c", c=WP)[:, :, 1:1 + W]

    with tile.TileContext(nc) as tc:
        with (
            tc.tile_pool(name="pers", bufs=1) as pers,
            tc.tile_pool(name="ps", bufs=8, space="PSUM") as psp,
            tc.tile_pool(name="gt", bufs=2) as gtp,
            tc.tile_pool(name="osb", bufs=2) as osbp,
        ):
            # --- persistent SBUF residents ---
            w0_t = pers.tile([2 * C, 5 * 4 * CH], cdt, tag="w0")
            u0_t = pers.tile([CH, 9 * 4 * CH], cdt, tag="u0")
            w1_t = pers.tile([CH, 9 * 4 * CH], cdt, tag="w1")
            u1_t = pers.tile([CH, 9 * 4 * CH], cdt, tag="u1")
            wt_t = pers.tile([CH, 9 * C], cdt, tag="wt")
            b0_t = pers.tile([CH, 4], f32, tag="b0")
            b1_t = pers.tile([CH, 4], f32, tag="b1")
            bt_t = pers.tile([C, 1], f32, tag="bt")
            xb66 = pers.tile([2 * C, BUFC], cdt, tag="xb66")
            xb2 = pers.tile([2 * C, BUFC], cdt, tag="xb2")
            h0p = [pers.tile([CH, BUFC], cdt, tag=f"h0p{i}", name=f"h0p{i}")
                   for i in range(2)]
            h1p = [pers.tile([CH, BUFC], cdt, tag=f"h1p{i}", name=f"h1p{i}")
                   for i in range(2)]
            c0_t = pers.tile([CH, HW], f32, tag="c0")
            c1_t = pers.tile([CH, HW], f32, tag="c1")

            for t_, d_ in ((w0_t, w0_d), (u0_t, u0_d), (w1_t, w1_d),
                           (u1_t, u1_d), (wt_t, wt_d), (b0_t, b0_d),
                           (b1_t, b1_d), (bt_t, bt_d)):
                nc.sync.dma_start(t_[:], d_.ap())

            # one-time zero fill (margins/padding stay zero forever; the
            # interiors are fully re-written by DMA/compute every iteration)
            for buf in (xb66, xb2, h0p[0], h0p[1], h1p[0], h1p[1]):
                if MM_DT == "f32r":
                    nc.sync.dma_start(buf[:], zz_d.ap()[:buf.shape[0]])
                else:
                    nc.vector.memset(buf[:], 0.0)

            def init_states():
                nc.sync.dma_start(interior(h0p[0], BASE, H), h0_d.ap())
                nc.sync.dma_start(interior(h1p[0], BASE, H), h1_d.ap())
                nc.sync.dma_start(c0_t[:], c0_d.ap())
                nc.sync.dma_start(c1_t[:], c1_d.ap())

            def l0_xtaps(g):
                # x-side matmul slots for layer 0: (lhs, src, np_rhs, off)
                res = []
                for k, (off, srcn, paired) in enumerate(L0SLOTS):
                    src = xb66 if srcn == "xb66" else xb2
                    o = k * 4 * CH + g * CH
                    kk = 2 * C if paired else C
                    res.append((w0_t[:kk, o:o + CH], src, kk, off))
                return res

    # taps for a standard 9-tap conv operand
            def std_taps(w_t, src, kx, g):
                res = []
                for ti in range(9):
                    dy, dx = TAPS[ti]
                    o = ti * 4 * CH + g * CH
                    res.append((w_t[:kx, o:o + CH], src, kx, dy * WP + dx))
                return res

            def conv_gates(xtaps_fn, hin, wh_t, b_t, c_t, hout, h_first):
                """One ConvLSTM cell; 5-chunk groups share stationary
                weights (5 matmuls per ldweights)."""
                for bi in range(0, len(CHUNKS), 2):
                    pair = CHUNKS[bi:bi + 2]
                    gtiles = [[None] * 4 for _ in pair]
                    for g in range(4):
                        pss = [psp.tile([CH, (r1 - r0) * WP], f32, tag="ps",
                                        name="ps") for (r0, r1) in pair]
                        xt = xtaps_fn(g)
                        ht = std_taps(wh_t, hin, CH, g)
                        taps = (ht + xt) if h_first else (xt + ht)
                        nt = len(taps)
                        for k, (lhs, src, kk, off) in enumerate(taps):
                            for j, (r0, r1) in enumerate(pair):
                                s = BASE + r0 * WP + off
                                cw = (r1 - r0) * WP
                                nc.tensor.matmul(pss[j][:], lhs,
                                                 src[:kk, s:s + cw],
                                                 start=(k == 0),
                                                 stop=(k == nt - 1))
                        for j, (r0, r1) in enumerate(pair):
                            nr = r1 - r0
                            gt = gtp.tile([CH, nr * W], f32, tag=f"g{g}",
                                          name=f"g{g}")
                            func = AF.Tanh if g == 2 else AF.Sigmoid
                            nc.scalar.activation(
                                gt[:].rearrange("p (r c) -> p r c", c=W),
                                pss[j][:].rearrange(
                                    "p (r c) -> p r c", c=WP)[:, :, 1:1 + W],
                                func, bias=b_t[:, g:g + 1])
                            gtiles[j][g] = gt
                    for j, (r0, r1) in enumerate(pair):
                        nr = r1 - r0
                        gi, gf, gg, go = gtiles[j]
                        csl = c_t[:, r0 * W:r1 * W]
                        nc.vector.tensor_mul(gg[:], gi[:], gg[:])   # i*g
                        nc.vector.tensor_mul(csl, gf[:], csl)       # f*c
                        nc.vector.tensor_add(csl, csl, gg[:])       # c
                        nc.scalar.activation(gf[:], csl, AF.Tanh)
                        nc.vector.tensor_mul(
                            interior(hout, BASE + r0 * WP, nr),
                            go[:].rearrange("p (r c) -> p r c", c=W),
                            gf[:].rearrange("p (r c) -> p r c", c=W))

            def conv_top(hin, tout):
                for bi in range(0, len(CHUNKS), 2):
                    pair = CHUNKS[bi:bi + 2]
                    pss = [psp.tile([C, (r1 - r0) * WP], f32, tag="ps",
                                    name="ps") for (r0, r1) in pair]
                    for ti in range(9):
                        dy, dx = TAPS[ti]
                        lhs = wt_t[:, ti * C:(ti + 1) * C]
                        for j, (r0, r1) in enumerate(pair):
                            s = BASE + r0 * WP + dy * WP + dx
                            cw = (r1 - r0) * WP
                            nc.tensor.matmul(pss[j][:], lhs, hin[:, s:s + cw],
                                             start=(ti == 0), stop=(ti == 8))
                    for j, (r0, r1) in enumerate(pair):
                        nr = r1 - r0
                        ot = osbp.tile([C, nr * W], f32, tag="ot", name="ot")
                        nc.scalar.activation(
                            ot[:].rearrange("p (r c) -> p r c", c=W),
                            pss[j][:].rearrange(
                                "p (r c) -> p r c", c=WP)[:, :, 1:1 + W],
                            AF.Identity, bias=bt_t[:, 0:1])
                        nc.sync.dma_start(tout[:, r0 * W:r1 * W], ot[:])

            def load_x(t):
                # x strip into: xb66[0:64]@BASE, xb66[64:128]@BASE-66,
                # xb2[0:64]@BASE, xb2[64:128]@BASE-2
                src = xs_d.ap()[t]
                nc.sync.dma_start(
                    interior(xb66[:C, :], BASE, H), src)
                nc.sync.dma_start(
                    interior(xb66[C:2 * C, :], BASE - 66, H), src)
                nc.sync.dma_start(
                    interior(xb2[:C, :], BASE, H), src)
                nc.sync.dma_start(
                    interior(xb2[C:2 * C, :], BASE - 2, H), src)

            def l1_xtaps_for(h0buf):
                return lambda g: std_taps(w1_t, h0buf, CH, g)

            def body():
                init_states()
                conv_top(h1p[0], out_d.ap()[0])
                for t in range(NSTEP):
                    load_x(t)
                    conv_gates(l0_xtaps, h0p[t % 2], u0_t, b0_t, c0_t,
                               h0p[(t + 1) % 2], h_first=False)
                    conv_gates(l1_xtaps_for(h0p[(t + 1) % 2]), h1p[t % 2],
                               u1_t, b1_t, c1_t, h1p[(t + 1) % 2],
                               h_first=True)
                    conv_top(h1p[(t + 1) % 2], out_d.ap()[t + 1])

            if LOOP_N > 0:
                with tc.For_i(0, LOOP_N, 1):
                    body()
            else:
                body()

    nc.compile()
    return nc


def _get_nc():
    if "nc" not in _CACHE:
        _CACHE["nc"] = _build()
    return _CACHE["nc"]


def kernel(target, h0, c0, h1, c1,
           wi0, bi0, wh0, bh0,
           wi1, bi1, wh1, bh1,
           wtop, btop):
    from concourse.bass_utils import run_bass_kernel_spmd

    nc = _get_nc()

    target = np.asarray(target, np.float32)
    shared = {
        "w0": _prep_w0(np.asarray(wi0, np.float32)),
        "u0": _prep_w(np.asarray(wh0, np.float32)),
        "w1": _prep_w(np.asarray(wi1, np.float32)),
        "u1": _prep_w(np.asarray(wh1, np.float32)),
        "wt": _prep_w(np.asarray(wtop, np.float32)),
        "b0": np.ascontiguousarray(
            (np.asarray(bi0) + np.asarray(bh0)).astype(np.float32)
            .reshape(4, CH).T),
        "b1": np.ascontiguousarray(
            (np.asarray(bi1) + np.asarray(bh1)).astype(np.float32)
            .reshape(4, CH).T),
        "bt": np.asarray(btop, np.float32).reshape(C, 1),
        "zz": _host_cast(np.zeros((CH, BUFC), np.float32)),
    }
    in_maps = []
    for b in range(B):
        m = dict(shared)
        m["xs"] = _host_cast(target[b, :NSTEP].reshape(NSTEP, C, HW))
        m["h0i"] = _host_cast(np.asarray(h0, np.float32)[b].reshape(CH, HW))
        m["c0i"] = np.ascontiguousarray(
            np.asarray(c0, np.float32)[b].reshape(CH, HW))
        m["h1i"] = _host_cast(np.asarray(h1, np.float32)[b].reshape(CH, HW))
        m["c1i"] = np.ascontiguousarray(
            np.asarray(c1, np.float32)[b].reshape(CH, HW))
        in_maps.append(m)

    res = run_bass_kernel_spmd(nc, in_maps, core_ids=list(range(B)))
    out = np.stack([res.results[b]["out"].reshape(T, C, H, W)
                    for b in range(B)])
    return out


# revision 10
# speedup vs baseline: 1.0680x; 1.0006x over previous
"""ConvLSTM decoder (2 ConvLSTM layers + top conv) on 8 Trainium2 cores.

Sharding: data-parallel over batch — B=8, one batch element per core,
weights replicated. The T=10 recurrence runs fully on-core.

Layout: images are stored in SBUF as a zero-padded flat row-major strip:
each 64-pixel row padded to 66 cols (1 zero col each side), 64 rows
contiguous, plus 68-col zero margins at both ends. A 3x3 'SAME' conv then
becomes 9 shifted matmuls accumulated in PSUM: for tap (dy,dx) the rhs is
the flat strip shifted by dy*66+dx.

Layer-0 i2h has only 64 input channels (half the PE array). Its 9 taps are
packed into 5 matmul groups of K=128 by stacking x with a shifted copy of
x in partitions 64:128 of two buffers (shift +66 pairs taps {-67,-1},
{-66,0},{-65,1}; shift +2 pairs {65,67}; tap 66 stays K=64).
"""

import numpy as np

B, T, C, H, W = 8, 10, 64, 64, 64
CH = 128
NSTEP = T - 1          # 9 recurrent steps
WP = W + 2             # padded row width
FLAT = H * WP          # 4224
MARG = 68              # >= 67 = max |tap offset|
BUFC = MARG + FLAT + MARG
BASE = MARG
HW = H * W             # 4096

# row chunks (r0, r1): 8x7 rows + 2x4 rows; max matmul N = 7*66 = 462 <= 512
CHUNKS = [(i * 7, i * 7 + 7) for i in range(8)] + [(56, 60), (60, 64)]
# weight-sharing groups: matmuls per ldweights = group size; <=4 banks open
CGROUPS = [(0, 4), (4, 8), (8, 10)]

TAPS = [(dy, dx) for dy in (-1, 0, 1) for dx in (-1, 0, 1)]

# layer-0 i2h tap packing: (offset_of_group0_tap, paired?) per slot;
# slots 0-2 pair (o, o+66) on xb66; slot 3 pairs (65, 67) on xb2;
# slot 4 is the lone K=64 tap at offset 66 read from xb66[0:64].
L0SLOTS = [(-67, "xb66", True), (-66, "xb66", True), (-65, "xb66", True),
           (65, "xb2", True), (66, "xb66", False)]
# (ky,kx) kernel indices per slot: group0 tap, group1 tap
L0SLOT_KK = [((0, 0), (1, 0)), ((0, 1), (1, 1)), ((0, 2), (1, 2)),
             ((2, 0), (2, 2)), ((2, 1), None)]

MM_DT = "bf16"         # "f32" | "f32r" | "bf16"
LOOP_N = 0             # >0: wrap body in a hardware repeat loop (timing only)

_CACHE = {}


def _np_dt(mybir):
    if MM_DT == "bf16":
        return mybir.dt.bfloat16
    if MM_DT == "f32r":
        return mybir.dt.float32r
    return mybir.dt.float32


def _host_cast(a):
    if MM_DT == "bf16":
        import ml_dtypes
        return np.ascontiguousarray(a.astype(ml_dtypes.bfloat16))
    return np.ascontiguousarray(a.astype(np.float32))


def _prep_w(w):
    # [O, I, 3, 3] -> [I, 9*O]; slice for (tap ti, 128-chunk g): ti*O + g*128
    O, I = w.shape[0], w.shape[1]
    return _host_cast(w.transpose(1, 2, 3, 0).reshape(I, 9 * O))


def _prep_w0(w):
    # [512, 64, 3, 3] -> [128, 5*512] slot-stacked for L0 i2h packing
    O, I = w.shape[0], w.shape[1]
    out = np.zeros((2 * I, 5 * O), np.float32)
    for k, (a, b) in enumerate(L0SLOT_KK):
        out[:I, k * O:(k + 1) * O] = w[:, :, a[0], a[1]].T
        if b is not None:
            out[I:2 * I, k * O:(k + 1) * O] = w[:, :, b[0], b[1]].T
    return _host_cast(out)


def _build():
    import concourse.bass as bass
    import concourse.tile as tile
    from concourse import bacc, mybir

    f32 = mybir.dt.float32
    cdt = _np_dt(mybir)          # matmul-input dtype in SBUF (and DRAM)
    AF = mybir.ActivationFunctionType

    nc = bacc.Bacc("TRN2", target_bir_lowering=False, debug=False,
                   num_devices=8)

    ddt = cdt if MM_DT in ("f32r", "bf16") else f32
    xs_d = nc.dram_tensor("xs", [NSTEP, C, HW], ddt, kind="ExternalInput")
    h0_d = nc.dram_tensor("h0i", [CH, HW], ddt, kind="ExternalInput")
    c0_d = nc.dram_tensor("c0i", [CH, HW], f32, kind="ExternalInput")
    h1_d = nc.dram_tensor("h1i", [CH, HW], ddt, kind="ExternalInput")
    c1_d = nc.dram_tensor("c1i", [CH, HW], f32, kind="ExternalInput")
    w0_d = nc.dram_tensor("w0", [2 * C, 5 * 4 * CH], ddt, kind="ExternalInput")
    u0_d = nc.dram_tensor("u0", [CH, 9 * 4 * CH], ddt, kind="ExternalInput")
    w1_d = nc.dram_tensor("w1", [CH, 9 * 4 * CH], ddt, kind="ExternalInput")
    u1_d = nc.dram_tensor("u1", [CH, 9 * 4 * CH], ddt, kind="ExternalInput")
    wt_d = nc.dram_tensor("wt", [CH, 9 * C], ddt, kind="ExternalInput")
    zz_d = nc.dram_tensor("zz", [CH, BUFC], ddt, kind="ExternalInput")
    b0_d = nc.dram_tensor("b0", [CH, 4], f32, kind="ExternalInput")
    b1_d = nc.dram_tensor("b1", [CH, 4], f32, kind="ExternalInput")
    bt_d = nc.dram_tensor("bt", [C, 1], f32, kind="ExternalInput")
    out_d = nc.dram_tensor("out", [T, C, HW], f32, kind="ExternalOutput")

    def interior(ap_2d, s0, nrow):
        # rows of 64 interior cols at stride 66 starting at flat offset s0
        return ap_2d[:, s0:s0 + nrow * WP].rearrange(
            "p (r c) -> p r c", c=WP)[:, :, 1:1 + W]

    with tile.TileContext(nc) as tc:
        with (
            tc.tile_pool(name="pers", bufs=1) as pers,
            tc.tile_pool(name="ps", bufs=8, space="PSUM") as psp,
            tc.tile_pool(name="gt", bufs=5) as gtp,
            tc.tile_pool(name="osb", bufs=5) as osbp,
        ):
            # --- persistent SBUF residents ---
            w0_t = pers.tile([2 * C, 5 * 4 * CH], cdt, tag="w0")
            u0_t = pers.tile([CH, 9 * 4 * CH], cdt, tag="u0")
            w1_t = pers.tile([CH, 9 * 4 * CH], cdt, tag="w1")
            u1_t = pers.tile([CH, 9 * 4 * CH], cdt, tag="u1")
            wt_t = pers.tile([CH, 9 * C], cdt, tag="wt")
            b0_t = pers.tile([CH, 4], f32, tag="b0")
            b1_t = pers.tile([CH, 4], f32, tag="b1")
            bt_t = pers.tile([C, 1], f32, tag="bt")
            xb66 = pers.tile([2 * C, BUFC], cdt, tag="xb66")
            xb2 = pers.tile([2 * C, BUFC], cdt, tag="xb2")
            h0p = [pers.tile([CH, BUFC], cdt, tag=f"h0p{i}", name=f"h0p{i}")
                   for i in range(2)]
            h1p = [pers.tile([CH, BUFC], cdt, tag=f"h1p{i}", name=f"h1p{i}")
                   for i in range(2)]
            c0_t = pers.tile([CH, HW], f32, tag="c0")
            c1_t = pers.tile([CH, HW], f32, tag="c1")

            for t_, d_ in ((w0_t, w0_d), (u0_t, u0_d), (w1_t, w1_d),
                           (u1_t, u1_d), (wt_t, wt_d), (b0_t, b0_d),
                           (b1_t, b1_d), (bt_t, bt_d)):
                nc.sync.dma_start(t_[:], d_.ap())

            # one-time zero fill (margins/padding stay zero forever; the
            # interiors are fully re-written by DMA/compute every iteration)
            for buf in (xb66, xb2, h0p[0], h0p[1], h1p[0], h1p[1]):
                if MM_DT == "f32r":
                    nc.sync.dma_start(buf[:], zz_d.ap()[:buf.shape[0]])
                else:
                    nc.vector.memset(buf[:], 0.0)

            def init_states():
                nc.sync.dma_start(interior(h0p[0], BASE, H), h0_d.ap())
                nc.sync.dma_start(interior(h1p[0], BASE, H), h1_d.ap())
                nc.sync.dma_start(c0_t[:], c0_d.ap())
                nc.sync.dma_start(c1_t[:], c1_d.ap())

            def l0_xtaps(g):
                # x-side matmul slots for layer 0: (lhs, src, np_rhs, off)
                res = []
                for k, (off, srcn, paired) in enumerate(L0SLOTS):
                    src = xb66 if srcn == "xb66" else xb2
                    o = k * 4 * CH + g * CH
                    kk = 2 * C if paired else C
                    res.append((w0_t[:kk, o:o + CH], src, kk, off))
                return res

    # taps for a standard 9-tap conv operand
            def std_taps(w_t, src, kx, g):
                res = []
                for ti in range(9):
                    dy, dx = TAPS[ti]
                    o = ti * 4 * CH + g * CH
                    res.append((w_t[:kx, o:o + CH], src, kx, dy * WP + dx))
                return res

            def conv_gates(xtaps_fn, hin, wh_t, b_t, c_t, hout, h_first):
                """One ConvLSTM cell; chunk groups share stationary
                weights (group-size matmuls per ldweights)."""
                for bi, be in CGROUPS:
                    pair = CHUNKS[bi:be]
                    gtiles = [[None] * 4 for _ in pair]
                    for g in range(4):
                        pss = [psp.tile([CH, (r1 - r0) * WP], f32, tag="ps",
                                        name="ps") for (r0, r1) in pair]
                        xt = xtaps_fn(g)
                        ht = std_taps(wh_t, hin, CH, g)
                        taps = (ht + xt) if h_first else (xt + ht)
                        nt = len(taps)
                        for k, (lhs, src, kk, off) in enumerate(taps):
                            for j, (r0, r1) in enumerate(pair):
                                s = BASE + r0 * WP + off
                                cw = (r1 - r0) * WP
                                nc.tensor.matmul(pss[j][:], lhs,
                                                 src[:kk, s:s + cw],
                                                 start=(k == 0),
                                                 stop=(k == nt - 1))
                        for j, (r0, r1) in enumerate(pair):
                            nr = r1 - r0
                            gt = gtp.tile([CH, nr * W], f32, tag=f"g{g}",
                                          name=f"g{g}")
                            func = AF.Tanh if g == 2 else AF.Sigmoid
                            nc.scalar.activation(
                                gt[:].rearrange("p (r c) -> p r c", c=W),
                                pss[j][:].rearrange(
                                    "p (r c) -> p r c", c=WP)[:, :, 1:1 + W],
                                func, bias=b_t[:, g:g + 1])
                            gtiles[j][g] = gt
                    for j, (r0, r1) in enumerate(pair):
                        nr = r1 - r0
                        gi, gf, gg, go = gtiles[j]
                        csl = c_t[:, r0 * W:r1 * W]
                        nc.vector.tensor_mul(gg[:], gi[:], gg[:])   # i*g
                        nc.vector.tensor_mul(csl, gf[:], csl)       # f*c
                        nc.vector.tensor_add(csl, csl, gg[:])       # c
                        nc.scalar.activation(gf[:], csl, AF.Tanh)
                        nc.vector.tensor_mul(
                            interior(hout, BASE + r0 * WP, nr),
                            go[:].rearrange("p (r c) -> p r c", c=W),
                            gf[:].rearrange("p (r c) -> p r c", c=W))

            def conv_top(hin, tout):
                for bi, be in CGROUPS:
                    pair = CHUNKS[bi:be]
                    pss = [psp.tile([C, (r1 - r0) * WP], f32, tag="ps",
                                    name="ps") for (r0, r1) in pair]
                    for ti in range(9):
                        dy, dx = TAPS[ti]
                        lhs = wt_t[:, ti * C:(ti + 1) * C]
                        for j, (r0, r1) in enumerate(pair):
                            s = BASE + r0 * WP + dy * WP + dx
                            cw = (r1 - r0) * WP
                            nc.tensor.matmul(pss[j][:], lhs, hin[:, s:s + cw],
                                             start=(ti == 0), stop=(ti == 8))
                    for j, (r0, r1) in enumerate(pair):
                        nr = r1 - r0
                        ot = osbp.tile([C, nr * W], f32, tag="ot", name="ot")
                        nc.scalar.activation(
                            ot[:].rearrange("p (r c) -> p r c", c=W),
                            pss[j][:].rearrange(
                                "p (r c) -> p r c", c=WP)[:, :, 1:1 + W],
                            AF.Identity, bias=bt_t[:, 0:1])
                        nc.sync.dma_start(tout[:, r0 * W:r1 * W], ot[:])

            def load_x(t):
                # x strip into: xb66[0:64]@BASE, xb66[64:128]@BASE-66,
                # xb2[0:64]@BASE, xb2[64:128]@BASE-2
                src = xs_d.ap()[t]
                nc.sync.dma_start(
                    interior(xb66[:C, :], BASE, H), src)
                nc.sync.dma_start(
                    interior(xb66[C:2 * C, :], BASE - 66, H), src)
                nc.sync.dma_start(
                    interior(xb2[:C, :], BASE, H), src)
                nc.sync.dma_start(
                    interior(xb2[C:2 * C, :], BASE - 2, H), src)

            def l1_xtaps_for(h0buf):
                return lambda g: std_taps(w1_t, h0buf, CH, g)

            def body():
                init_states()
                conv_top(h1p[0], out_d.ap()[0])
                for t in range(NSTEP):
                    load_x(t)
                    conv_gates(l0_xtaps, h0p[t % 2], u0_t, b0_t, c0_t,
                               h0p[(t + 1) % 2], h_first=False)
                    conv_gates(l1_xtaps_for(h0p[(t + 1) % 2]), h1p[t % 2],
                               u1_t, b1_t, c1_t, h1p[(t + 1) % 2],
                               h_first=True)
                    conv_top(h1p[(t + 1) % 2], out_d.ap()[t + 1])

            if LOOP_N > 0:
                with tc.For_i(0, LOOP_N, 1):
                    body()
            else:
                body()

    nc.compile()
    return nc


def _get_nc():
    if "nc" not in _CACHE:
        _CACHE["nc"] = _build()
    return _CACHE["nc"]


def kernel(target, h0, c0, h1, c1,
           wi0, bi0, wh0, bh0,
           wi1, bi1, wh1, bh1,
           wtop, btop):
    from concourse.bass_utils import run_bass_kernel_spmd

    nc = _get_nc()

    target = np.asarray(target, np.float32)
    shared = {
        "w0": _prep_w0(np.asarray(wi0, np.float32)),
        "u0": _prep_w(np.asarray(wh0, np.float32)),
        "w1": _prep_w(np.asarray(wi1, np.float32)),
        "u1": _prep_w(np.asarray(wh1, np.float32)),
        "wt": _prep_w(np.asarray(wtop, np.float32)),
        "b0": np.ascontiguousarray(
            (np.asarray(bi0) + np.asarray(bh0)).astype(np.float32)
            .reshape(4, CH).T),
        "b1": np.ascontiguousarray(
            (np.asarray(bi1) + np.asarray(bh1)).astype(np.float32)
            .reshape(4, CH).T),
        "bt": np.asarray(btop, np.float32).reshape(C, 1),
        "zz": _host_cast(np.zeros((CH, BUFC), np.float32)),
    }
    in_maps = []
    for b in range(B):
        m = dict(shared)
        m["xs"] = _host_cast(target[b, :NSTEP].reshape(NSTEP, C, HW))
        m["h0i"] = _host_cast(np.asarray(h0, np.float32)[b].reshape(CH, HW))
        m["c0i"] = np.ascontiguousarray(
            np.asarray(c0, np.float32)[b].reshape(CH, HW))
        m["h1i"] = _host_cast(np.asarray(h1, np.float32)[b].reshape(CH, HW))
        m["c1i"] = np.ascontiguousarray(
            np.asarray(c1, np.float32)[b].reshape(CH, HW))
        in_maps.append(m)

    res = run_bass_kernel_spmd(nc, in_maps, core_ids=list(range(B)))
    out = np.stack([res.results[b]["out"].reshape(T, C, H, W)
                    for b in range(B)])
    return out
